# revision 26
# baseline (speedup 1.0000x reference)
"""Trainium2 Bass kernel for the CapsuleLayer routing problem.

Strategy (8 NeuronCores, shard the input-capsule dim I):
  - Each core owns I_loc = 256 input capsules; votes[b, i_loc, d, a] are
    computed on the TensorEngine with block-diagonal x as the stationary
    operand and the capsule weights streamed from HBM once (4 MB/core bf16),
    then kept in SBUF as bf16 in layout [partition=(j, b16), free=(g, a, d)]
    (i = 8*g + j).
  - Routing runs fully on-chip.  The two batch halves (h = 0, 1) are
    INDEPENDENT chains; they are emitted staggered with per-half AllReduces
    so one half's collective/squash latency hides under the other half's
    DVE work (the DVE is the bottleneck engine).
  - The iter-0 uniform-route preactivation accumulates straight off the
    weight stream via h-merged matmuls (lhsT = [128, 32] covering both
    halves), so its AllReduces fire during the PREVIOUS rep's routing.
  - Squash is fused onto the Scalar engine where possible: for a constant
    bias tensor (the nn.Module uses nn.init.constant_) the bias-add rides
    the Square activation's bias port; t2 = preact + bias is recomputed on
    the idle Pool engine off the critical path.
  - PSUM->SBUF vote copies are pair-batched ([128, 2*O] per copy) on the
    Scalar engine, emitted only during DVE-heavy routing phases so they
    never queue ahead of the critical squash activations.
  - The Pool engine also steals one quarter of the first logits update
    (u = V * asum for h1/q1) whose deadline is late enough to absorb
    Pool's ~3.5x lower elementwise throughput.
  - The final iteration's partials go straight to HBM and the host does the
    last bias+squash in fp64.
  - V is double-buffered across benchmark reps so rep r+1's votes production
    (DMA/PE/ACT) pipelines under rep r's routing (DVE).
"""

import functools

import numpy as np
import ml_dtypes

import concourse.bass as bass
import concourse.tile as tile
from concourse import bacc, mybir
from concourse import bass_utils

N_CORES = 8
B, I, C, D, A = 32, 2048, 16, 32, 16
I_LOC = I // N_CORES          # 256 capsules per core
G = I_LOC // 8                # 32 groups of 8 capsules
GH = G // 2                   # V is split into two g-range tiles per half
O = A * D                     # 512, free layout is (a, d) with d innermost

F32 = mybir.dt.float32
BF16 = mybir.dt.bfloat16
_nbf16 = ml_dtypes.bfloat16


def _build(num_routing: int, reps: int = 1, opts: frozenset = frozenset()):
    nc = bacc.Bacc("TRN2", target_bir_lowering=False, debug=False,
                   enable_asserts=True, num_devices=N_CORES)
    const_bias = "genbias" not in opts

    w_in = nc.dram_tensor("w", [I_LOC * C, O], BF16, kind="ExternalInput").ap()
    xd_in = nc.dram_tensor("xd", [G, 2, 128, 128], BF16, kind="ExternalInput").ap()
    sh_in = nc.dram_tensor("sh", [128, 16], BF16, kind="ExternalInput").ap()
    xt_in = nc.dram_tensor("xt", [G, 128, 2, 16], BF16, kind="ExternalInput").ap()
    bsc_in = nc.dram_tensor("bsc", [128, 1], F32, kind="ExternalInput").ap()
    if not const_bias:
        bias_in = nc.dram_tensor("biasb", [128, O], F32, kind="ExternalInput").ap()
    outp = nc.dram_tensor("outp", [B, O], F32, kind="ExternalOutput").ap()

    Exp = mybir.ActivationFunctionType.Exp
    Square = mybir.ActivationFunctionType.Square
    Sqrt = mybir.ActivationFunctionType.Sqrt
    Copy = mybir.ActivationFunctionType.Copy
    add = mybir.AluOpType.add
    mult = mybir.AluOpType.mult
    AX = mybir.AxisListType.X
    nbuf = 2 if reps > 1 else 1

    with tile.TileContext(nc) as tc:
        with (
            tc.tile_pool(name="persist", bufs=1) as persist,
            tc.tile_pool(name="wpool", bufs=2) as wpool,
            tc.tile_pool(name="wapool", bufs=2) as wapool,
            tc.tile_pool(name="xpool", bufs=2) as xpool,
            tc.tile_pool(name="pspool", bufs=2, space="PSUM") as pspool,
            tc.tile_pool(name="papool", bufs=1, space="PSUM") as papool,
            tc.tile_pool(name="padpool", bufs=2, space="PSUM") as padpool,
            tc.tile_pool(name="stage", bufs=1) as stage,
            tc.tile_pool(name="rpool", bufs=1) as rpool,
            tc.tile_pool(name="upool", bufs=1) as upool,
            tc.tile_pool(name="wvcpool", bufs=2) as wvcpool,
            tc.tile_pool(name="stlpool", bufs=1) as stlpool,
            tc.tile_pool(name="small", bufs=2) as small,
            tc.tile_pool(name="dram", bufs=4, space="DRAM") as dram,
        ):
            # V[buf][h][q]: votes for batch-half h, g in [q*GH, (q+1)*GH);
            # double-buffered over reps so votes(rep+1) overlaps routing(rep)
            V = [[[persist.tile([128, GH, A, D], BF16, tag=f"V{p}{h}{q}",
                                name=f"V{p}{h}{q}") for q in range(2)]
                  for h in range(2)] for p in range(nbuf)]
            # bf16 logits: |L| < ~8 and the vote dot-products are bf16-sourced
            # anyway; 16-bit keeps the DVE ops in 2x mode
            L = [persist.tile([128, G, D], BF16, tag=f"L{h}", name=f"L{h}")
                 for h in range(2)]
            asum = [persist.tile([128, A, D], BF16, tag=f"as{h}",
                                 name=f"as{h}") for h in range(2)]
            sh_sb = persist.tile([128, 16], BF16, tag="sh", name="sh_sb")
            xt_sb = persist.tile([128, G, 2, 16], BF16, tag="xt", name="xt_sb")
            bsc_sb = persist.tile([128, 1], F32, tag="bsc", name="bsc_sb")
            nc.sync.dma_start(sh_sb[:], sh_in[:])
            nc.sync.dma_start(xt_sb[:], xt_in.rearrange("g p h b -> p g h b"))
            nc.sync.dma_start(bsc_sb[:], bsc_in[:])
            if not const_bias:
                bias_sb = persist.tile([128, A, D], F32, tag="bias",
                                       name="bias_sb")
                nc.sync.dma_start(bias_sb[:],
                                  bias_in.rearrange("p (a d) -> p a d", a=A))

            def make_paD(rep):
                """Phase A of votes production: the iter-0 uniform-route
                preactivation (h-merged, M=32) off its own weight stream,
                emitted at rep start while the PE is otherwise idle, so the
                NEXT rep's iter-0 AllReduce can fire mid-rep."""
                paD = padpool.tile([32, O], F32, tag="paD", name="paD")
                for gp in range(G // 2):
                    wt = wapool.tile([128, 2, O], BF16, tag="wtA", name="wtA")
                    nc.sync.dma_start(
                        wt[:], w_in[bass.ts(gp, 256), :]
                        .rearrange("(t p) o -> p t o", p=128))
                    for gg in range(2):
                        g = 2 * gp + gg
                        nc.tensor.matmul(
                            paD[:],
                            lhsT=xt_sb[:, g].rearrange("p h b -> p (h b)"),
                            rhs=wt[:, gg], start=(g == 0), stop=(g == G - 1))
                return paD

            def make_votes_emitters(rep):
                """Phase B of votes production for `rep` as per-g-pair
                closures, emitted interleaved through the PREVIOUS rep's
                routing phase so the per-engine streams overlap votes
                DMA/PE/ACT with routing DVE."""
                Vc = V[rep % nbuf]

                def emit_pair(gp):
                    # paired-g transfers halve the DMA descriptor count (the
                    # sync queue's per-descriptor issue rate gates the votes
                    # pipeline otherwise)
                    wt = wpool.tile([128, 2, O], BF16, tag="wt", name="wt")
                    nc.sync.dma_start(
                        wt[:], w_in[bass.ts(gp, 256), :]
                        .rearrange("(t p) o -> p t o", p=128))
                    xdt = xpool.tile([128, 2, 2, 128], BF16, tag="xdt",
                                     name="xdt")
                    nc.sync.dma_start(
                        xdt[:], xd_in[bass.ts(gp, 2)]
                        .rearrange("t h p m -> p t h m"))
                    for h in range(2):
                        ps = pspool.tile([128, 2, O], F32, tag="ps", name="ps")
                        for gg in range(2):
                            nc.tensor.matmul(ps[:, gg],
                                             lhsT=xdt[:, gg, h, :],
                                             rhs=wt[:, gg], start=True,
                                             stop=True)
                        g0 = 2 * gp
                        dst = Vc[h][g0 // GH][:, g0 % GH:g0 % GH + 2]
                        nc.scalar.copy(dst[:], ps[:].rearrange(
                            "p t (a d) -> p t a d", a=A))

                return [functools.partial(emit_pair, gp)
                        for gp in range(G // 2)]

            def pop_emitters(n):
                for _ in range(n):
                    if pending:
                        pending.pop(0)()

            def fire_ar(h, it, src):
                """DMA a bf16 preactivation to DRAM and AllReduce it."""
                inb = dram.tile([16, O], BF16, tag=f"ari{h}", name="arin")
                outb = dram.tile([16, O], BF16, tag=f"aro{h}", name="arout",
                                 addr_space="Shared")
                nc.sync.dma_start(inb[:], src)
                if "nocc" in opts:
                    nc.sync.dma_start(outb[:], inb[:])
                else:
                    nc.gpsimd.collective_compute(
                        "AllReduce", add,
                        replica_groups=[list(range(N_CORES))],
                        ins=[inb[:].opt()], outs=[outb[:].opt()])
                return outb

            def bcast_prep(h, outb):
                # issued from the ACT queue: the sync queue is ~80 DMAs deep
                # per rep and would delay these ~10us past data-ready
                prep = stage.tile([128, A, D], BF16, tag=f"pp{h}",
                                  name="prep")
                for j in range(8):
                    nc.scalar.dma_start(
                        prep[bass.ts(j, 16)].rearrange("b a d -> b (a d)"),
                        outb[:])
                return prep

            def fire_ar0_bcast(paD):
                """Iter-0 AllReduce + partition broadcast for a rep; called
                mid-way through the PREVIOUS rep so everything lands well
                before the rep boundary."""
                pre2 = small.tile([32, O], BF16, tag="pre2", name="pre2")
                nc.scalar.copy(pre2[:], paD[:])
                preps = []
                for h in range(2):
                    outb = fire_ar(h, 0, pre2[bass.ts(h, 16)])
                    preps.append(bcast_prep(h, outb))
                return preps

            def squash(h, it, prep):
                """Per-half squash on the broadcast preactivation:
                act = (preact+bias) * |n|/(1+n^2) into asum[h] (it==0
                writes, else accumulates)."""
                # norm chain: sq is written (d, a)-transposed so the norm
                # reduce is a unit-stride innermost reduction on the DVE
                sq = stage.tile([128, A, D], BF16, tag="sq", name="sq")
                t2 = stage.tile([128, A, D], BF16, tag=f"t2{h}", name="t2")
                if const_bias:
                    # bias-add rides the Square's bias port (ACT); t2 is
                    # recomputed on the DVE in parallel
                    nc.scalar.activation(sq[:], prep[:], Square,
                                         bias=bsc_sb[:, 0:1])
                    nc.vector.tensor_scalar_add(t2[:], prep[:],
                                                bsc_sb[:, 0:1])
                else:
                    nc.vector.tensor_tensor(t2[:], prep[:], bias_sb[:], add)
                    nc.scalar.activation(sq[:], t2[:], Square)
                n2 = small.tile([128, D], F32, tag=f"n2{h}", name="n2")
                nc.vector.tensor_reduce(n2[:], sq[:].rearrange("p a d -> p d a"),
                                        axis=AX, op=add)
                nrm = small.tile([128, D], F32, tag=f"nr{h}", name="nrm")
                nc.scalar.activation(nrm[:], n2[:], Sqrt)
                den = small.tile([128, D], F32, tag=f"de{h}", name="den")
                nc.scalar.activation(den[:], n2[:], Copy, bias=1.0)
                rc2 = small.tile([128, D], F32, tag=f"rc{h}", name="rc2")
                nc.vector.reciprocal_approx_fast(out=rc2[:], in_=den[:])
                fac = small.tile([128, D], BF16, tag=f"fa{h}", name="fac")
                nc.vector.tensor_tensor(fac[:], nrm[:], rc2[:], mult)
                if it == 0:
                    nc.vector.tensor_tensor(
                        asum[h][:], t2[:],
                        fac[:, None, :].to_broadcast([128, A, D]), mult)
                else:
                    actb = stage.tile([128, A, D], BF16, tag="ab",
                                      name="actb")
                    nc.vector.tensor_tensor(
                        actb[:], t2[:],
                        fac[:, None, :].to_broadcast([128, A, D]), mult)
                    nc.vector.tensor_tensor(asum[h][:], asum[h][:], actb[:],
                                            add)

            def logits_part(h, gstart, n, Vc, eng, tag):
                """L[h][:, gstart:gstart+n] = sum_a V * asum (logits are
                linear in the accumulated activation asum, so L is recomputed
                fresh each iteration).  `eng` picks the engine (DVE, or Pool
                for the stolen part)."""
                q, go = gstart // GH, gstart % GH
                pool = {"u": upool, "wvc": wvcpool, "stl": stlpool}[tag]
                u = pool.tile([128, n, A, D], BF16, tag=tag, name="u")
                eng.tensor_tensor(
                    u[:], Vc[h][q][:, go:go + n],
                    asum[h][:, None, :, :].to_broadcast([128, n, A, D]),
                    mult)
                half = A // 2
                while half > 1:
                    eng.tensor_tensor(u[:, :, 0:half], u[:, :, 0:half],
                                      u[:, :, half:2 * half], add)
                    half //= 2
                eng.tensor_tensor(L[h][:, gstart:gstart + n],
                                  u[:, :, 0, :], u[:, :, 1, :], add)

            def softmax(h):
                """R = softmax over d of L[h]; |L| < ~8 so exp is safe in
                fp32 without the max-subtraction."""
                ex = stage.tile([128, G, D], BF16, tag="ex", name="ex")
                nc.scalar.activation(ex[:], L[h][:], Exp)
                sm = small.tile([128, G], F32, tag=f"sm{h}", name="sm")
                nc.vector.tensor_reduce(sm[:], ex[:], axis=AX, op=add)
                rc = small.tile([128, G], F32, tag=f"rz{h}", name="rc")
                nc.vector.reciprocal_approx_fast(out=rc[:], in_=sm[:])
                R = rpool.tile([128, G, D], BF16, tag="R", name="R")
                nc.vector.tensor_tensor(
                    R[:], ex[:],
                    rc[:, :, None].to_broadcast([128, G, D]), mult)
                return R

            def route_reduce(h, R, Vc):
                """pa[h] = sum_i route * V via DVE product + PE partition
                reduction against the 0/1 selection matrix sh."""
                pa = papool.tile([16, O], F32, tag=f"pa{h}", name=f"pa{h}")
                WVC = 8  # g-groups per wv chunk
                for q in range(2):
                    for cb in range(GH // WVC):
                        wv = wvcpool.tile([128, WVC, A, D], BF16,
                                          tag="wvc", name="wv")
                        gb = q * GH + cb * WVC
                        nc.vector.tensor_tensor(
                            wv[:], Vc[h][q][:, bass.ts(cb, WVC)],
                            R[:, gb:gb + WVC, None, :]
                            .to_broadcast([128, WVC, A, D]), mult)
                        for gg in range(WVC):
                            g = gb + gg
                            nc.tensor.matmul(pa[:], lhsT=sh_sb[:],
                                             rhs=wv[:, gg],
                                             start=(g == 0),
                                             stop=(g == G - 1))
                        pop_emitters(1)
                return pa[:]

            paD_cur = make_paD(0)
            pending = make_votes_emitters(0)
            pop_emitters(G // 2)  # prologue: first rep's votes up front
            preps0 = fire_ar0_bcast(paD_cur) if num_routing > 1 else None

            for _rep in range(reps):
              Vc = V[_rep % nbuf]
              paD = paD_cur
              if _rep + 1 < reps:
                  paD_cur = make_paD(_rep + 1)
                  pending = make_votes_emitters(_rep + 1)
              else:
                  pending = []

              # ---- routing iterations, h-staggered ----
              if num_routing == 1:
                  pref32 = persist.tile([32, O], F32, tag="pf", name="pref")
                  nc.scalar.copy(pref32[:], paD[:])
                  for h in range(2):
                      nc.sync.dma_start(outp[bass.ts(h, 16), :],
                                        pref32[bass.ts(h, 16)])
                  pop_emitters(G // 2)
                  continue

              # iter-0 preacts were AllReduced and broadcast during the
              # previous rep (prologue for rep 0)
              preps = preps0

              for it in range(num_routing - 1):
                  # h0 chain first; h1's squash/u runs on DVE while h0's
                  # AllReduce (next iter) is in flight, and vice versa
                  for h in range(2):
                      squash(h, it, preps[h])
                      pop_emitters(1)
                      # Pool steals h1/q1 of the FIRST logits update: its
                      # softmax deadline is ~35us after emission, enough for
                      # Pool's lower throughput; later quarters stay on DVE
                      # (their deadlines are tight).
                      steal = (it == 0 and h == 1 and num_routing > 2
                               and "nosteal" not in opts)
                      logits_part(h, 0, GH, Vc, nc.vector, "u")
                      pop_emitters(1)
                      if steal:
                          logits_part(h, GH, 8, Vc, nc.vector, "wvc")
                          logits_part(h, GH + 8, 8, Vc, nc.gpsimd, "stl")
                      else:
                          logits_part(h, GH, GH, Vc, nc.vector, "u")
                      pop_emitters(1)
                  nxt_last = it + 1 == num_routing - 1
                  for h in range(2):
                      Rh = softmax(h)
                      pa = route_reduce(h, Rh, Vc)
                      if nxt_last:
                          pref = persist.tile([16, O], F32, tag="pf2",
                                              name="pref")
                          nc.scalar.copy(pref[:], pa)
                          nc.sync.dma_start(outp[bass.ts(h, 16), :], pref[:])
                      else:
                          pre = small.tile([16, O], BF16, tag=f"pre{h}",
                                           name="pre")
                          nc.scalar.copy(pre[:], pa)
                          preps[h] = bcast_prep(h, fire_ar(h, it + 1, pre[:]))
                  if it == 0 and _rep + 1 < reps:
                      preps0 = fire_ar0_bcast(paD_cur)

              pop_emitters(G // 2)  # drain any leftover emitters

    nc.compile()
    return nc


KERNEL_OPTS = frozenset()


@functools.lru_cache(maxsize=4)
def _get_compiled(num_routing: int, opts: frozenset = KERNEL_OPTS):
    return _build(num_routing, opts=opts)


def _host_inputs(x, weights, opts: frozenset = frozenset()):
    """Build the per-core input maps (everything except tiny constants)."""
    x_np = np.ascontiguousarray(x.reshape(B, I, C), dtype=np.float32)
    # o' = a*D + d ordering
    w2 = np.ascontiguousarray(
        weights.reshape(I, C, D, A).transpose(0, 1, 3, 2), dtype=np.float32)
    x_np = x_np.astype(_nbf16)
    w2 = w2.astype(_nbf16)

    in_maps = []
    for r in range(N_CORES):
        sl = slice(r * I_LOC, (r + 1) * I_LOC)
        w_r = w2[sl].reshape(I_LOC * C, O)
        # xd[g, h, j*16+c, j*16+bh] = x[h*16+bh, r*I_LOC + g*8 + j, c]
        arr = x_np[:, sl, :].reshape(2, 16, G, 8, C)  # (h, bh, g, j, c)
        xd = np.zeros((G, 2, 128, 128), _nbf16)
        for j in range(8):
            xd[:, :, j * 16:(j + 1) * 16, j * 16:(j + 1) * 16] = \
                arr[:, :, :, j, :].transpose(2, 0, 3, 1)  # (g, h, c, bh)
        # xt[g, (j, c), h, bh] = x[h*16+bh, r*I_LOC + g*8 + j, c] / D
        xt = np.ascontiguousarray(
            (arr.astype(np.float32) / D).transpose(2, 3, 4, 0, 1)
            .reshape(G, 128, 2, 16)).astype(_nbf16)
        in_maps.append({"w": np.ascontiguousarray(w_r), "xd": xd, "xt": xt})
    return in_maps


def _host_constants(bias):
    # sh[j*16+bh, bh'] = (bh == bh')
    sh = np.zeros((128, 16), np.float32)
    for j in range(8):
        for bh in range(16):
            sh[j * 16 + bh, bh] = 1.0
    b0 = float(np.asarray(bias).flat[0])
    bsc = np.full((128, 1), b0, np.float32)
    return {"sh": sh.astype(_nbf16), "bsc": bsc}


def _host_constants_general(bias):
    consts = _host_constants(bias)
    bias2 = np.ascontiguousarray(
        bias.reshape(D, A).T, dtype=np.float32).reshape(O)
    consts["biasb"] = np.tile(bias2[None, :], (128, 1)).astype(np.float32)
    return consts


def _squash_host(t):
    # t: [B, D, A] float64; squash over a
    n2 = (t ** 2).sum(axis=2, keepdims=True)
    n = np.sqrt(n2)
    return t * (n / (1.0 + n2))


def kernel(x, weights, bias, num_routing):
    n = int(num_routing)
    x = np.asarray(x, dtype=np.float32)
    weights = np.asarray(weights, dtype=np.float32)
    bias_np = np.asarray(bias, dtype=np.float32)

    const_bias = bool(np.all(bias_np == bias_np.flat[0]))
    opts = KERNEL_OPTS if const_bias else (KERNEL_OPTS | {"genbias"})
    nc = _get_compiled(n, opts)
    in_maps = _host_inputs(x, weights, opts=opts)
    consts = (_host_constants(bias_np) if const_bias
              else _host_constants_general(bias_np))
    for m in in_maps:
        m.update(consts)

    # the axon tunnel occasionally returns a transient
    # NRT_EXEC_UNIT_UNRECOVERABLE; one retry has recovered every observed case
    import time as _time
    try:
        res = bass_utils.run_bass_kernel_spmd(
            nc, in_maps, core_ids=list(range(N_CORES)))
    except Exception:
        _time.sleep(10)
        res = bass_utils.run_bass_kernel_spmd(
            nc, in_maps, core_ids=list(range(N_CORES)))

    partials = np.stack([res.results[r]["outp"] for r in range(N_CORES)], axis=0)
    pre = partials.astype(np.float64).sum(axis=0)            # [B, O] in (a, d)
    pre = pre.reshape(B, A, D).transpose(0, 2, 1)            # [B, D, A]
    pre = pre + bias_np.reshape(D, A)[None].astype(np.float64)
    act = _squash_host(pre).astype(np.float32)
    return act.reshape(B, D, A, 1, 1)


if __name__ == "__main__":
    import sys
    sys.path.insert(0, "/root/problem")
    from reference import setup_inputs, reference

    inputs = {k: np.asarray(v) if not isinstance(v, int) else v
              for k, v in setup_inputs().items()}
    ref = np.asarray(reference(**inputs))
    out = kernel(**inputs)
    d = np.abs(out - ref)
    print("absmax", d.max(), "ref absmax", np.abs(ref).max(),
          "scale-rel", d.max() / np.abs(ref).max(),
          "rel_l2", np.linalg.norm(d) / np.linalg.norm(ref))


# revision 27
# speedup vs baseline: 1.0054x; 1.0054x over previous
"""Trainium2 Bass kernel for the CapsuleLayer routing problem.

Strategy (8 NeuronCores, shard the input-capsule dim I):
  - Each core owns I_loc = 256 input capsules; votes[b, i_loc, d, a] are
    computed on the TensorEngine with block-diagonal x as the stationary
    operand and the capsule weights streamed from HBM once (4 MB/core bf16),
    then kept in SBUF as bf16 in layout [partition=(j, b16), free=(g, a, d)]
    (i = 8*g + j).
  - Routing runs fully on-chip.  The two batch halves (h = 0, 1) are
    INDEPENDENT chains; they are emitted staggered with per-half AllReduces
    so one half's collective/squash latency hides under the other half's
    DVE work (the DVE is the bottleneck engine).
  - The iter-0 uniform-route preactivation accumulates straight off the
    weight stream via h-merged matmuls (lhsT = [128, 32] covering both
    halves), so its AllReduces fire during the PREVIOUS rep's routing.
  - Squash is fused onto the Scalar engine where possible: for a constant
    bias tensor (the nn.Module uses nn.init.constant_) the bias-add rides
    the Square activation's bias port; t2 = preact + bias is recomputed on
    the idle Pool engine off the critical path.
  - PSUM->SBUF vote copies are pair-batched ([128, 2*O] per copy) on the
    Scalar engine, emitted only during DVE-heavy routing phases so they
    never queue ahead of the critical squash activations.
  - The Pool engine also steals one quarter of the first logits update
    (u = V * asum for h1/q1) whose deadline is late enough to absorb
    Pool's ~3.5x lower elementwise throughput.
  - The final iteration's partials go straight to HBM and the host does the
    last bias+squash in fp64.
  - V is double-buffered across benchmark reps so rep r+1's votes production
    (DMA/PE/ACT) pipelines under rep r's routing (DVE).
"""

import functools

import numpy as np
import ml_dtypes

import concourse.bass as bass
import concourse.tile as tile
from concourse import bacc, mybir
from concourse import bass_utils

N_CORES = 8
B, I, C, D, A = 32, 2048, 16, 32, 16
I_LOC = I // N_CORES          # 256 capsules per core
G = I_LOC // 8                # 32 groups of 8 capsules
GH = G // 2                   # V is split into two g-range tiles per half
O = A * D                     # 512, free layout is (a, d) with d innermost

F32 = mybir.dt.float32
BF16 = mybir.dt.bfloat16
_nbf16 = ml_dtypes.bfloat16


def _build(num_routing: int, reps: int = 1, opts: frozenset = frozenset()):
    nc = bacc.Bacc("TRN2", target_bir_lowering=False, debug=False,
                   enable_asserts=True, num_devices=N_CORES)
    const_bias = "genbias" not in opts

    w_in = nc.dram_tensor("w", [I_LOC * C, O], BF16, kind="ExternalInput").ap()
    xd_in = nc.dram_tensor("xd", [G, 2, 128, 128], BF16, kind="ExternalInput").ap()
    sh_in = nc.dram_tensor("sh", [128, 16], BF16, kind="ExternalInput").ap()
    xt_in = nc.dram_tensor("xt", [G, 128, 2, 16], BF16, kind="ExternalInput").ap()
    bsc_in = nc.dram_tensor("bsc", [128, 1], F32, kind="ExternalInput").ap()
    if not const_bias:
        bias_in = nc.dram_tensor("biasb", [128, O], F32, kind="ExternalInput").ap()
    outp = nc.dram_tensor("outp", [B, O], F32, kind="ExternalOutput").ap()

    Exp = mybir.ActivationFunctionType.Exp
    Square = mybir.ActivationFunctionType.Square
    Sqrt = mybir.ActivationFunctionType.Sqrt
    Copy = mybir.ActivationFunctionType.Copy
    add = mybir.AluOpType.add
    mult = mybir.AluOpType.mult
    AX = mybir.AxisListType.X
    nbuf = 2 if reps > 1 else 1

    with tile.TileContext(nc) as tc:
        with (
            tc.tile_pool(name="persist", bufs=1) as persist,
            tc.tile_pool(name="wpool", bufs=2) as wpool,
            tc.tile_pool(name="xpool", bufs=2) as xpool,
            tc.tile_pool(name="pspool", bufs=2, space="PSUM") as pspool,
            tc.tile_pool(name="papool", bufs=1, space="PSUM") as papool,
            tc.tile_pool(name="padpool", bufs=2, space="PSUM") as padpool,
            tc.tile_pool(name="stage", bufs=1) as stage,
            tc.tile_pool(name="rpool", bufs=1) as rpool,
            tc.tile_pool(name="upool", bufs=1) as upool,
            tc.tile_pool(name="wvcpool", bufs=2) as wvcpool,
            tc.tile_pool(name="stlpool", bufs=1) as stlpool,
            tc.tile_pool(name="small", bufs=2) as small,
            tc.tile_pool(name="dram", bufs=4, space="DRAM") as dram,
        ):
            # V[buf][h][q]: votes for batch-half h, g in [q*GH, (q+1)*GH);
            # double-buffered over reps so votes(rep+1) overlaps routing(rep)
            V = [[[persist.tile([128, GH, A, D], BF16, tag=f"V{p}{h}{q}",
                                name=f"V{p}{h}{q}") for q in range(2)]
                  for h in range(2)] for p in range(nbuf)]
            # bf16 logits: |L| < ~8 and the vote dot-products are bf16-sourced
            # anyway; 16-bit keeps the DVE ops in 2x mode
            L = [persist.tile([128, G, D], BF16, tag=f"L{h}", name=f"L{h}")
                 for h in range(2)]
            asum = [persist.tile([128, A, D], BF16, tag=f"as{h}",
                                 name=f"as{h}") for h in range(2)]
            sh_sb = persist.tile([128, 16], BF16, tag="sh", name="sh_sb")
            xt_sb = persist.tile([128, G, 2, 16], BF16, tag="xt", name="xt_sb")
            bsc_sb = persist.tile([128, 1], F32, tag="bsc", name="bsc_sb")
            nc.sync.dma_start(sh_sb[:], sh_in[:])
            nc.sync.dma_start(xt_sb[:], xt_in.rearrange("g p h b -> p g h b"))
            nc.sync.dma_start(bsc_sb[:], bsc_in[:])
            if not const_bias:
                bias_sb = persist.tile([128, A, D], F32, tag="bias",
                                       name="bias_sb")
                nc.sync.dma_start(bias_sb[:],
                                  bias_in.rearrange("p (a d) -> p a d", a=A))

            def make_votes_emitters(rep):
                """Votes production for `rep` as per-g-pair closures, emitted
                interleaved through the PREVIOUS rep's routing phase so the
                per-engine streams overlap votes DMA/PE/ACT with routing
                DVE."""
                Vc = V[rep % nbuf]
                # h-merged iter-0 accumulator: partitions (h, bh) = 32
                paD = padpool.tile([32, O], F32, tag="paD", name="paD")

                def emit_pair(gp):
                    # paired-g transfers halve the DMA descriptor count (the
                    # sync queue's per-descriptor issue rate gates the votes
                    # pipeline otherwise)
                    wt = wpool.tile([128, 2, O], BF16, tag="wt", name="wt")
                    nc.sync.dma_start(
                        wt[:], w_in[bass.ts(gp, 256), :]
                        .rearrange("(t p) o -> p t o", p=128))
                    xdt = xpool.tile([128, 2, 2, 128], BF16, tag="xdt",
                                     name="xdt")
                    nc.sync.dma_start(
                        xdt[:], xd_in[bass.ts(gp, 2)]
                        .rearrange("t h p m -> p t h m"))
                    for gg in range(2):
                        g = 2 * gp + gg
                        # iter-0 preact accumulates straight off the weight
                        # stream, both halves in one matmul (M=32): no
                        # dependency on the V copies, so the next rep's
                        # collective input is ready early and its AllReduces
                        # execute hidden under this rep's DVE phases
                        nc.tensor.matmul(
                            paD[:],
                            lhsT=xt_sb[:, g].rearrange("p h b -> p (h b)"),
                            rhs=wt[:, gg], start=(g == 0), stop=(g == G - 1))
                    for h in range(2):
                        ps = pspool.tile([128, 2, O], F32, tag="ps", name="ps")
                        for gg in range(2):
                            nc.tensor.matmul(ps[:, gg],
                                             lhsT=xdt[:, gg, h, :],
                                             rhs=wt[:, gg], start=True,
                                             stop=True)
                        g0 = 2 * gp
                        dst = Vc[h][g0 // GH][:, g0 % GH:g0 % GH + 2]
                        nc.scalar.copy(dst[:], ps[:].rearrange(
                            "p t (a d) -> p t a d", a=A))

                return paD, [functools.partial(emit_pair, gp)
                             for gp in range(G // 2)]

            def pop_emitters(n):
                for _ in range(n):
                    if pending:
                        pending.pop(0)()

            def fire_ar(h, it, src):
                """DMA a bf16 preactivation to DRAM and AllReduce it."""
                inb = dram.tile([16, O], BF16, tag=f"ari{h}", name="arin")
                outb = dram.tile([16, O], BF16, tag=f"aro{h}", name="arout",
                                 addr_space="Shared")
                nc.sync.dma_start(inb[:], src)
                if "nocc" in opts:
                    nc.sync.dma_start(outb[:], inb[:])
                else:
                    nc.gpsimd.collective_compute(
                        "AllReduce", add,
                        replica_groups=[list(range(N_CORES))],
                        ins=[inb[:].opt()], outs=[outb[:].opt()])
                return outb

            def squash(h, it, outb):
                """Per-half squash: broadcast the reduced preact to all 128
                partitions, then act = (preact+bias) * |n|/(1+n^2) into
                asum[h] (it==0 writes, else accumulates)."""
                prep = stage.tile([128, A, D], BF16, tag=f"pp{h}",
                                  name="prep")
                for j in range(8):
                    nc.sync.dma_start(
                        prep[bass.ts(j, 16)].rearrange("b a d -> b (a d)"),
                        outb[:])
                # norm chain: sq is written (d, a)-transposed so the norm
                # reduce is a unit-stride innermost reduction on the DVE
                sq = stage.tile([128, A, D], BF16, tag="sq", name="sq")
                t2 = stage.tile([128, A, D], BF16, tag=f"t2{h}", name="t2")
                if const_bias:
                    # bias-add rides the Square's bias port (ACT); t2 is
                    # recomputed on the DVE in parallel
                    nc.scalar.activation(sq[:], prep[:], Square,
                                         bias=bsc_sb[:, 0:1])
                    nc.vector.tensor_scalar_add(t2[:], prep[:],
                                                bsc_sb[:, 0:1])
                else:
                    nc.vector.tensor_tensor(t2[:], prep[:], bias_sb[:], add)
                    nc.scalar.activation(sq[:], t2[:], Square)
                n2 = small.tile([128, D], F32, tag=f"n2{h}", name="n2")
                nc.vector.tensor_reduce(n2[:], sq[:].rearrange("p a d -> p d a"),
                                        axis=AX, op=add)
                nrm = small.tile([128, D], F32, tag=f"nr{h}", name="nrm")
                nc.scalar.activation(nrm[:], n2[:], Sqrt)
                den = small.tile([128, D], F32, tag=f"de{h}", name="den")
                nc.scalar.activation(den[:], n2[:], Copy, bias=1.0)
                rc2 = small.tile([128, D], F32, tag=f"rc{h}", name="rc2")
                nc.vector.reciprocal_approx_fast(out=rc2[:], in_=den[:])
                fac = small.tile([128, D], BF16, tag=f"fa{h}", name="fac")
                nc.vector.tensor_tensor(fac[:], nrm[:], rc2[:], mult)
                if it == 0:
                    nc.vector.tensor_tensor(
                        asum[h][:], t2[:],
                        fac[:, None, :].to_broadcast([128, A, D]), mult)
                else:
                    actb = stage.tile([128, A, D], BF16, tag="ab",
                                      name="actb")
                    nc.vector.tensor_tensor(
                        actb[:], t2[:],
                        fac[:, None, :].to_broadcast([128, A, D]), mult)
                    nc.vector.tensor_tensor(asum[h][:], asum[h][:], actb[:],
                                            add)

            def logits_part(h, gstart, n, Vc, eng, tag):
                """L[h][:, gstart:gstart+n] = sum_a V * asum (logits are
                linear in the accumulated activation asum, so L is recomputed
                fresh each iteration).  `eng` picks the engine (DVE, or Pool
                for the stolen part)."""
                q, go = gstart // GH, gstart % GH
                pool = {"u": upool, "wvc": wvcpool, "stl": stlpool}[tag]
                u = pool.tile([128, n, A, D], BF16, tag=tag, name="u")
                eng.tensor_tensor(
                    u[:], Vc[h][q][:, go:go + n],
                    asum[h][:, None, :, :].to_broadcast([128, n, A, D]),
                    mult)
                half = A // 2
                while half > 1:
                    eng.tensor_tensor(u[:, :, 0:half], u[:, :, 0:half],
                                      u[:, :, half:2 * half], add)
                    half //= 2
                eng.tensor_tensor(L[h][:, gstart:gstart + n],
                                  u[:, :, 0, :], u[:, :, 1, :], add)

            def softmax(h):
                """R = softmax over d of L[h]; |L| < ~8 so exp is safe in
                fp32 without the max-subtraction."""
                ex = stage.tile([128, G, D], BF16, tag="ex", name="ex")
                nc.scalar.activation(ex[:], L[h][:], Exp)
                sm = small.tile([128, G], F32, tag=f"sm{h}", name="sm")
                nc.vector.tensor_reduce(sm[:], ex[:], axis=AX, op=add)
                rc = small.tile([128, G], F32, tag=f"rz{h}", name="rc")
                nc.vector.reciprocal_approx_fast(out=rc[:], in_=sm[:])
                R = rpool.tile([128, G, D], BF16, tag="R", name="R")
                nc.vector.tensor_tensor(
                    R[:], ex[:],
                    rc[:, :, None].to_broadcast([128, G, D]), mult)
                return R

            def route_reduce(h, R, Vc):
                """pa[h] = sum_i route * V via DVE product + PE partition
                reduction against the 0/1 selection matrix sh."""
                pa = papool.tile([16, O], F32, tag=f"pa{h}", name=f"pa{h}")
                WVC = 8  # g-groups per wv chunk
                for q in range(2):
                    for cb in range(GH // WVC):
                        wv = wvcpool.tile([128, WVC, A, D], BF16,
                                          tag="wvc", name="wv")
                        gb = q * GH + cb * WVC
                        nc.vector.tensor_tensor(
                            wv[:], Vc[h][q][:, bass.ts(cb, WVC)],
                            R[:, gb:gb + WVC, None, :]
                            .to_broadcast([128, WVC, A, D]), mult)
                        for gg in range(WVC):
                            g = gb + gg
                            nc.tensor.matmul(pa[:], lhsT=sh_sb[:],
                                             rhs=wv[:, gg],
                                             start=(g == 0),
                                             stop=(g == G - 1))
                        pop_emitters(1)
                return pa[:]

            paD_next, pending = make_votes_emitters(0)
            pop_emitters(G // 2)  # prologue: first rep's votes up front

            for _rep in range(reps):
              Vc = V[_rep % nbuf]
              paD = paD_next
              if _rep + 1 < reps:
                  paD_next, pending = make_votes_emitters(_rep + 1)
              else:
                  pending = []

              # ---- routing iterations, h-staggered ----
              if num_routing == 1:
                  pref32 = persist.tile([32, O], F32, tag="pf", name="pref")
                  nc.scalar.copy(pref32[:], paD[:])
                  for h in range(2):
                      nc.sync.dma_start(outp[bass.ts(h, 16), :],
                                        pref32[bass.ts(h, 16)])
                  pop_emitters(G // 2)
                  continue

              # iter-0 preacts come from paD ([32, O], both halves); one
              # 32-partition copy, then per-half AllReduces
              pre2 = small.tile([32, O], BF16, tag="pre2", name="pre2")
              nc.scalar.copy(pre2[:], paD[:])
              outbs = [fire_ar(h, 0, pre2[bass.ts(h, 16)]) for h in range(2)]

              for it in range(num_routing - 1):
                  # h0 chain first; h1's squash/u runs on DVE while h0's
                  # AllReduce (next iter) is in flight, and vice versa
                  for h in range(2):
                      squash(h, it, outbs[h])
                      pop_emitters(1)
                      # Pool steals h1/q1 of the FIRST logits update: its
                      # softmax deadline is ~35us after emission, enough for
                      # Pool's lower throughput; later quarters stay on DVE
                      # (their deadlines are tight).
                      steal = (it == 0 and h == 1 and num_routing > 2
                               and "nosteal" not in opts)
                      logits_part(h, 0, GH, Vc, nc.vector, "u")
                      pop_emitters(1)
                      if steal:
                          logits_part(h, GH, 8, Vc, nc.vector, "wvc")
                          logits_part(h, GH + 8, 8, Vc, nc.gpsimd, "stl")
                      else:
                          logits_part(h, GH, GH, Vc, nc.vector, "u")
                      pop_emitters(1)
                  nxt_last = it + 1 == num_routing - 1
                  for h in range(2):
                      Rh = softmax(h)
                      pa = route_reduce(h, Rh, Vc)
                      if nxt_last:
                          pref = persist.tile([16, O], F32, tag="pf2",
                                              name="pref")
                          nc.scalar.copy(pref[:], pa)
                          nc.sync.dma_start(outp[bass.ts(h, 16), :], pref[:])
                      else:
                          pre = small.tile([16, O], BF16, tag=f"pre{h}",
                                           name="pre")
                          nc.scalar.copy(pre[:], pa)
                          outbs[h] = fire_ar(h, it + 1, pre[:])

              pop_emitters(G // 2)  # drain any leftover emitters

    nc.compile()
    return nc


KERNEL_OPTS = frozenset()


@functools.lru_cache(maxsize=4)
def _get_compiled(num_routing: int, opts: frozenset = KERNEL_OPTS):
    return _build(num_routing, opts=opts)


def _host_inputs(x, weights, opts: frozenset = frozenset()):
    """Build the per-core input maps (everything except tiny constants)."""
    x_np = np.ascontiguousarray(x.reshape(B, I, C), dtype=np.float32)
    # o' = a*D + d ordering
    w2 = np.ascontiguousarray(
        weights.reshape(I, C, D, A).transpose(0, 1, 3, 2), dtype=np.float32)
    x_np = x_np.astype(_nbf16)
    w2 = w2.astype(_nbf16)

    in_maps = []
    for r in range(N_CORES):
        sl = slice(r * I_LOC, (r + 1) * I_LOC)
        w_r = w2[sl].reshape(I_LOC * C, O)
        # xd[g, h, j*16+c, j*16+bh] = x[h*16+bh, r*I_LOC + g*8 + j, c]
        arr = x_np[:, sl, :].reshape(2, 16, G, 8, C)  # (h, bh, g, j, c)
        xd = np.zeros((G, 2, 128, 128), _nbf16)
        for j in range(8):
            xd[:, :, j * 16:(j + 1) * 16, j * 16:(j + 1) * 16] = \
                arr[:, :, :, j, :].transpose(2, 0, 3, 1)  # (g, h, c, bh)
        # xt[g, (j, c), h, bh] = x[h*16+bh, r*I_LOC + g*8 + j, c] / D
        xt = np.ascontiguousarray(
            (arr.astype(np.float32) / D).transpose(2, 3, 4, 0, 1)
            .reshape(G, 128, 2, 16)).astype(_nbf16)
        in_maps.append({"w": np.ascontiguousarray(w_r), "xd": xd, "xt": xt})
    return in_maps


def _host_constants(bias):
    # sh[j*16+bh, bh'] = (bh == bh')
    sh = np.zeros((128, 16), np.float32)
    for j in range(8):
        for bh in range(16):
            sh[j * 16 + bh, bh] = 1.0
    b0 = float(np.asarray(bias).flat[0])
    bsc = np.full((128, 1), b0, np.float32)
    return {"sh": sh.astype(_nbf16), "bsc": bsc}


def _host_constants_general(bias):
    consts = _host_constants(bias)
    bias2 = np.ascontiguousarray(
        bias.reshape(D, A).T, dtype=np.float32).reshape(O)
    consts["biasb"] = np.tile(bias2[None, :], (128, 1)).astype(np.float32)
    return consts


def _squash_host(t):
    # t: [B, D, A] float64; squash over a
    n2 = (t ** 2).sum(axis=2, keepdims=True)
    n = np.sqrt(n2)
    return t * (n / (1.0 + n2))


def kernel(x, weights, bias, num_routing):
    n = int(num_routing)
    x = np.asarray(x, dtype=np.float32)
    weights = np.asarray(weights, dtype=np.float32)
    bias_np = np.asarray(bias, dtype=np.float32)

    const_bias = bool(np.all(bias_np == bias_np.flat[0]))
    opts = KERNEL_OPTS if const_bias else (KERNEL_OPTS | {"genbias"})
    nc = _get_compiled(n, opts)
    in_maps = _host_inputs(x, weights, opts=opts)
    consts = (_host_constants(bias_np) if const_bias
              else _host_constants_general(bias_np))
    for m in in_maps:
        m.update(consts)

    # the axon tunnel occasionally returns a transient
    # NRT_EXEC_UNIT_UNRECOVERABLE; one retry has recovered every observed case
    import time as _time
    try:
        res = bass_utils.run_bass_kernel_spmd(
            nc, in_maps, core_ids=list(range(N_CORES)))
    except Exception:
        _time.sleep(10)
        res = bass_utils.run_bass_kernel_spmd(
            nc, in_maps, core_ids=list(range(N_CORES)))

    partials = np.stack([res.results[r]["outp"] for r in range(N_CORES)], axis=0)
    pre = partials.astype(np.float64).sum(axis=0)            # [B, O] in (a, d)
    pre = pre.reshape(B, A, D).transpose(0, 2, 1)            # [B, D, A]
    pre = pre + bias_np.reshape(D, A)[None].astype(np.float64)
    act = _squash_host(pre).astype(np.float32)
    return act.reshape(B, D, A, 1, 1)


if __name__ == "__main__":
    import sys
    sys.path.insert(0, "/root/problem")
    from reference import setup_inputs, reference

    inputs = {k: np.asarray(v) if not isinstance(v, int) else v
              for k, v in setup_inputs().items()}
    ref = np.asarray(reference(**inputs))
    out = kernel(**inputs)
    d = np.abs(out - ref)
    print("absmax", d.max(), "ref absmax", np.abs(ref).max(),
          "scale-rel", d.max() / np.abs(ref).max(),
          "rel_l2", np.linalg.norm(d) / np.linalg.norm(ref))
